# revision 58
# baseline (speedup 1.0000x reference)
"""GF(2) linear block encoder c = (b @ G) mod 2 on 8 TRN2 NeuronCores.

Strategy:
  - Data-parallel: shard b rows (32768 -> 8 x 4096), replicate G.
  - Bits {0,1} are exact in fp8-e4m3 and products accumulate exactly in
    fp32 PSUM, so the GF(2) matmul is an fp8 DoubleRow matmul (K=256 per
    MM). HW floor: 216ns per 512-col DR matmul (1 col/cycle @2.4GHz),
    512 MMs/core = 110.7us of PE streaming.
  - Extraction: ACT casts PSUM fp32 -> uint16, DVE ands with 1 and
    casts to uint8 rows staged in SBUF; host upcasts to int32.
  - DMA reality (measured): queues process ~25-50 descriptors/us each
    (contended across all 8 cores), one descriptor per partition per
    piece; descriptors up to ~8KB carry more bytes for the same count.
    Queue starts: sync ~8.2us, scalar ~8.6, gpsimd ~10.0.
  - Head: the ~640-descriptor critical set (b chunks 0-1 + all four
    kp h0 G pieces) is partition-split into 64-descriptor pieces across
    the three queues in exact consumption order; a kp-outer block over
    m-tiles 0-2 consumes G pieces as they land. Zeroed 512-col warmup
    matmuls hold the PE p-state at 2.4GHz through the supply window.
  - Output: m-tiles 0-9 run per-phase (G h1 hasn't landed yet) and ship
    1KB-descriptor half rows; m-tiles 10-31 run BOTH phases fused and
    ship one full 2KB-descriptor row each - descriptor demand stays
    ~37/us instead of crunching 4096 descriptors into phase 1. The last
    two rows leave partition-split so no queue holds a 128-descriptor
    piece at program end.
"""

import sys

import numpy as np

if "/opt/trn_rl_repo" not in sys.path:
    sys.path.insert(0, "/opt/trn_rl_repo")

import ml_dtypes

B_ROWS = 32768
K_MSG = 1024
N_CODE = 2048
NCORES = 8
M = B_ROWS // NCORES  # 4096 rows per core
KS = K_MSG // 128     # 8 k-subtiles of 128
KP = KS // 2          # 4 DoubleRow k-pair steps (K=256 each)
MT = M // 128         # 32 m-tiles
MC = 16               # b chunks along m (2 m-tiles each)
MCW = M // MC         # 256 rows per chunk
BG = 4                # b chunks per group tile
NBG = MC // BG        # 4 groups
HMT = 3               # head-block m-tiles (kp-outer, 3 PSUM half-tiles)
SPLIT_MT = 10         # m-tiles processed per-phase before fusing

F8 = ml_dtypes.float8_e4m3

_NC_CACHE = None


def _build_bass():
    import concourse.bacc as bacc
    import concourse.mybir as mybir
    from concourse import tile

    nc = bacc.Bacc("TRN2", target_bir_lowering=False, debug=False)

    # bt[p, c, s, j] = b bit for row m = c*MCW + j, k = s*128 + p
    bt = nc.dram_tensor("bt", [128, MC, KS, MCW], mybir.dt.float8e4, kind="ExternalInput")
    # g[p, kp, h, r, j] = G bit for k = (2*kp + r)*128 + p, n = h*1024 + j
    g = nc.dram_tensor("g", [128, KP, 2, 2, 1024], mybir.dt.float8e4, kind="ExternalInput")
    # output is PACKED (2 bits/byte, byte j holds n=2j and n=2j+1) and
    # P-MAJOR ([p, mt, j]) so multi-row pieces are contiguous per
    # partition: the whole 4MB leaves in ~5 dmas of 128 descriptors.
    # The host transposes/unpacks.
    c = nc.dram_tensor("c", [128, MT, N_CODE // 2], mybir.dt.uint8, kind="ExternalOutput")

    dr = mybir.MatmulPerfMode.DoubleRow
    NH = N_CODE // 2

    with tile.TileContext(nc) as tc:
        with (
            tc.tile_pool(name="persist", bufs=1) as persist,
            tc.tile_pool(name="psum", bufs=4, space="PSUM") as psum_pool,
            tc.tile_pool(name="mids", bufs=8) as mids,
            tc.tile_pool(name="tmps", bufs=8) as tmps,
        ):
            # g_tiles[kp][p, h, r, j]
            g_tiles = [
                persist.tile([128, 2, 2, 1024], mybir.dt.float8e4, name=f"gt{kp}", tag=f"g{kp}")
                for kp in range(KP)
            ]
            b_groups = [
                persist.tile([128, BG, KS, MCW], mybir.dt.float8e4, name=f"bg{i}", tag=f"bg{i}")
                for i in range(NBG)
            ]

            def gh_part(kp, h, p0, p1, eng):
                # partition range of one (kp, n-half) G piece (2KB descs)
                eng.dma_start(out=g_tiles[kp][p0:p1, h], in_=g[p0:p1, kp, h])

            def bpair_part(ch, p0, p1, eng):
                # partition range of a chunk-PAIR (4KB descriptors)
                gi, sl = ch // BG, ch % BG
                eng.dma_start(
                    out=b_groups[gi][p0:p1, sl : sl + 2],
                    in_=bt[p0:p1, ch : ch + 2],
                )

            # --- input pushes: strict consumption order, partition-split
            # for the critical set so each queue's early pieces are only
            # 64 descriptors deep; b rides as chunk-pairs (4KB descs);
            # G h1 pieces come right after the critical set (needed from
            # the phase-1 revisit of m-tiles 0-9, ~30us).
            # sync (starts ~8.2us)
            gh_part(0, 0, 0, 64, nc.sync)
            gh_part(0, 0, 64, 128, nc.sync)
            gh_part(1, 0, 64, 128, nc.sync)
            gh_part(2, 0, 0, 64, nc.sync)
            bpair_part(2, 0, 128, nc.sync)     # chunks 2-3 (mt4-7)
            gh_part(0, 1, 0, 128, nc.sync)
            bpair_part(6, 0, 128, nc.sync)     # chunks 6-7 (mt12-15)
            gh_part(2, 1, 0, 128, nc.sync)
            bpair_part(8, 0, 128, nc.sync)     # chunks 8-9 (mt16-19)
            bpair_part(12, 0, 128, nc.sync)    # chunks 12-13 (mt24-27)
            # scalar (starts ~8.6us; issues done ~14.5, free for ACTs)
            bpair_part(0, 0, 64, nc.scalar)    # chunks 0-1 (mt0-3)
            bpair_part(0, 64, 128, nc.scalar)
            gh_part(2, 0, 64, 128, nc.scalar)
            gh_part(3, 0, 64, 128, nc.scalar)
            # gpsimd (starts ~10.0us)
            gh_part(1, 0, 0, 64, nc.gpsimd)
            gh_part(3, 0, 0, 64, nc.gpsimd)
            bpair_part(4, 0, 128, nc.gpsimd)   # chunks 4-5 (mt8-11)
            gh_part(1, 1, 0, 128, nc.gpsimd)
            gh_part(3, 1, 0, 128, nc.gpsimd)
            bpair_part(10, 0, 128, nc.gpsimd)  # chunks 10-11 (mt20-23)
            bpair_part(14, 0, 128, nc.gpsimd)  # chunks 14-15 (mt28-31)

            # --- PE warmups on zeroed dummy tiles into a dedicated PSUM
            # tile: no data deps beyond the early DVE memsets, so they run
            # from ~7.1us and hold the DVFS ramp until real data lands.
            zw0 = persist.tile([128, 2, 128], mybir.dt.float8e4, name="zw0")
            zw = persist.tile([128, 2, 512], mybir.dt.float8e4, name="zwarm")
            nc.vector.memset(zw0, 0)
            nc.vector.memset(zw, 0)
            ps_warm = psum_pool.tile([128, NH], mybir.dt.float32, name="ps")

            def warm(cols=512):
                src = zw0 if cols <= 128 else zw
                nc.tensor.matmul(
                    ps_warm[:, 0:cols],
                    src[:, :, 0:128],
                    src[:, :, 0:cols],
                    start=True,
                    stop=True,
                    perf_mode=dr,
                )

            for _ in range(4):
                warm(64)
            for _ in range(7):
                warm(512)

            # packed output staging, same [p, mt, j] layout as the DRAM
            # output, so multi-row pieces are contiguous on both sides
            NPK = N_CODE // 2
            HPK = NPK // 2
            c8all = persist.tile([128, MT, NPK], mybir.dt.uint8, name="c8all")

            def bsta(mt, kp):
                mc, j = mt // 2, mt % 2
                return b_groups[mc // BG][
                    :, mc % BG, 2 * kp : 2 * kp + 2, j * 128 : (j + 1) * 128
                ]

            def grhs(kp, ph, q):
                # [128, 2, 512] moving operand: n-cols ph*1024+q*512 ..+512
                return g_tiles[kp][:, ph, :, q * 512 : (q + 1) * 512]

            def extract(mid, cdst, ps, m0, m1, o0, o1):
                # PSUM fp32 -> uint16 (ACT cast), DVE masks parities and
                # packs adjacent columns two-per-byte: (odd&1)*2 + (even&1).
                # o0:o1 are PACKED byte offsets ((m1-m0)/2 wide).
                w = (m1 - m0) // 2
                nc.scalar.activation(
                    mid[:, m0:m1], ps, mybir.ActivationFunctionType.Copy
                )
                nc.vector.tensor_scalar(
                    out=mid[:, m0:m1], in0=mid[:, m0:m1], scalar1=1,
                    scalar2=None, op0=mybir.AluOpType.bitwise_and,
                )
                ta = tmps.tile([128, NH // 2], mybir.dt.uint16)
                nc.vector.tensor_scalar(
                    out=ta[:, 0:w], in0=mid[:, m0 + 1 : m1 : 2], scalar1=2,
                    scalar2=None, op0=mybir.AluOpType.mult,
                )
                nc.vector.tensor_tensor(
                    out=cdst[:, o0:o1], in0=ta[:, 0:w], in1=mid[:, m0:m1:2],
                    op=mybir.AluOpType.add,
                )

            def mm_half(ps, mt, kp, ph):
                for nt in range(2):
                    nc.tensor.matmul(
                        ps[:, nt * 512 : (nt + 1) * 512],
                        bsta(mt, kp),
                        grhs(kp, ph, nt),
                        start=(kp == 0),
                        stop=(kp == KP - 1),
                        perf_mode=dr,
                    )

            def ship_rows(t0, t1, eng, p0=0, p1=128):
                # rows [t0, t1) leave as one dma: (t1-t0) KB contiguous per
                # partition on both sides
                eng.dma_start(out=c[p0:p1, t0:t1], in_=c8all[p0:p1, t0:t1])

            # --- phase-0 head block: m-tiles 0..2, kp-OUTER so each G h0
            # piece is consumed the moment it lands; seam warmups after the
            # first two kp rounds absorb arrival jitter.
            head_ps = [
                psum_pool.tile([128, NH], mybir.dt.float32, name="ps")
                for _ in range(HMT)
            ]
            for kp in range(KP):
                for hm in range(HMT):
                    mm_half(head_ps[hm], hm, kp, 0)
                if kp < 2:
                    warm(512)
            for hm in range(HMT):
                mid = mids.tile([128, NH], mybir.dt.uint16)
                extract(mid, c8all[:, hm], head_ps[hm], 0, NH, 0, HPK)

            def do_half(mt, ph):
                ps = psum_pool.tile([128, NH], mybir.dt.float32, name="ps")
                for kp in range(KP):
                    mm_half(ps, mt, kp, ph)
                mid = mids.tile([128, NH], mybir.dt.uint16)
                extract(mid, c8all[:, mt], ps, 0, NH, ph * HPK, (ph + 1) * HPK)

            # --- per-phase stretch while G h1 is still arriving
            for mt in range(HMT, SPLIT_MT):
                do_half(mt, 0)
            for mt in range(SPLIT_MT):
                do_half(mt, 1)

            # --- fused stretch: both halves back-to-back; completed rows
            # leave in 8-row groups (8KB contiguous per partition on both
            # sides = 128 descriptors per MB)
            for mt in range(SPLIT_MT, MT):
                last2 = mt >= MT - 2
                do_half(mt, 0)
                if not last2:
                    do_half(mt, 1)
                else:
                    # per-bank PSUM quarters for the final half so the tail
                    # is one 512-col extract chain
                    mid = mids.tile([128, NH], mybir.dt.uint16)
                    for nt in range(2):
                        psq = psum_pool.tile([128, 512], mybir.dt.float32, name="ps")
                        for kp in range(KP):
                            nc.tensor.matmul(
                                psq,
                                bsta(mt, kp),
                                grhs(kp, 1, nt),
                                start=(kp == 0),
                                stop=(kp == KP - 1),
                                perf_mode=dr,
                            )
                        m0, m1 = nt * 512, (nt + 1) * 512
                        extract(mid, c8all[:, mt], psq, m0, m1,
                                HPK + nt * 256, HPK + (nt + 1) * 256)
                if mt == SPLIT_MT:
                    ship_rows(0, 8, nc.sync)
                elif mt == 15:
                    ship_rows(8, 16, nc.gpsimd)
                elif mt == 23:
                    ship_rows(16, 24, nc.sync)
                elif mt == 29:
                    ship_rows(24, 30, nc.gpsimd)
                elif mt == MT - 1:
                    # final two rows: three 43-descriptor pieces
                    ship_rows(30, 32, nc.sync, 0, 43)
                    ship_rows(30, 32, nc.scalar, 43, 86)
                    ship_rows(30, 32, nc.gpsimd, 86, 128)


    nc.finalize()
    return nc


def _get_nc():
    global _NC_CACHE
    if _NC_CACHE is None:
        _NC_CACHE = _build_bass()
    return _NC_CACHE


def _pack_inputs(b, G):
    b8 = np.asarray(b).astype(np.uint8)
    G8 = np.asarray(G).astype(np.uint8)
    # g[p, kp, h, r, j]: k = (2*kp + r)*128 + p, n = h*1024 + j
    g_psn = G8.reshape(KS, 128, N_CODE).transpose(1, 0, 2)   # [p, s, n]
    g_f8 = (
        g_psn.reshape(128, KP, 2, 2, 1024)                    # [p, kp, r, h, j]
        .transpose(0, 1, 3, 2, 4)                             # [p, kp, h, r, j]
        .astype(F8, order="C")
    )
    bts = []
    for core in range(NCORES):
        sh = b8[core * M : (core + 1) * M]  # [M, K]
        # bt[p, c, s, j]: m = c*MCW + j, k = s*128 + p
        btc = sh.reshape(MC, MCW, KS, 128).transpose(3, 0, 2, 1)
        bts.append(btc.astype(F8, order="C"))
    return bts, g_f8


def kernel(b, G, trace=False, **run_kwargs):
    from concourse.bass_utils import run_bass_kernel_spmd

    nc = _get_nc()
    bts, g_f8 = _pack_inputs(b, G)
    in_maps = [{"bt": bts[i], "g": g_f8} for i in range(NCORES)]
    res = run_bass_kernel_spmd(
        nc, in_maps, core_ids=list(range(NCORES)), trace=trace, **run_kwargs
    )
    # per-core output is [p, mt, j] packed 2 bits/byte; unpack+reorder:
    # row m = mt*128 + p, bits n=2j (bit 0) and n=2j+1 (bit 1)
    out = np.empty((B_ROWS, N_CODE), dtype=np.int32)
    for i in range(NCORES):
        pk = res.results[i]["c"].transpose(1, 0, 2)  # [mt, p, j]
        sl = out[i * M : (i + 1) * M].reshape(MT, 128, N_CODE)
        sl[:, :, 0::2] = pk & 1
        sl[:, :, 1::2] = (pk >> 1) & 1
    if trace:
        kernel.last_results = res
    return out


kernel.last_results = None


# revision 59
# speedup vs baseline: 1.0728x; 1.0728x over previous
"""GF(2) linear block encoder c = (b @ G) mod 2 on 8 TRN2 NeuronCores.

Strategy:
  - Data-parallel: shard b rows (32768 -> 8 x 4096), replicate G.
  - Bits {0,1} are exact in fp8-e4m3 and products accumulate exactly in
    fp32 PSUM, so the GF(2) matmul is an fp8 DoubleRow matmul (K=256 per
    MM). HW floor: 216ns per 512-col DR matmul (1 col/cycle @2.4GHz),
    512 MMs/core = 110.6us of PE streaming.
  - Output is uint8 bits (ACT casts PSUM fp32 -> uint16, DVE ands with
    1 and casts to uint8), upcast to int32 on the host.
  - Head: the framework preamble ends ~6.6us; each dma_start costs
    ~0.65us of issue time and first data lands ~8.2us. Critical pieces
    (b chunk 0 split in half on the otherwise-idle scalar queue, G
    512-col quarter slices striped kp-wise across sync/gpsimd) are
    pushed first so m-tile 0 can start ~8.6us. 512-col zeroed
    warmup matmuls bridge 6.8->8.6us and drive the DVFS ramp (PE runs
    at 1.2GHz until ~3us of sustained load); single warmups fill the
    two early data seams.
  - Tail: last two m-tiles extract per 512-col PSUM bank so the final
    chain is one quarter extract + one 64KiB DMA on emptied queues.
"""

import sys

import numpy as np

if "/opt/trn_rl_repo" not in sys.path:
    sys.path.insert(0, "/opt/trn_rl_repo")

import ml_dtypes

B_ROWS = 32768
K_MSG = 1024
N_CODE = 2048
NCORES = 8
M = B_ROWS // NCORES  # 4096 rows per core
KS = K_MSG // 128     # 8 k-subtiles of 128
KP = KS // 2          # 4 DoubleRow k-pair steps (K=256 each)
MT = M // 128         # 32 m-tiles
MC = 16               # b chunks along m (2 m-tiles each)
MCW = M // MC         # 256 rows per chunk
BG = 4                # b chunks per group tile
NBG = MC // BG        # 4 groups

F8 = ml_dtypes.float8_e4m3

_NC_CACHE = None


def _build_bass():
    import concourse.bacc as bacc
    import concourse.mybir as mybir
    from concourse import tile

    nc = bacc.Bacc("TRN2", target_bir_lowering=False, debug=False)

    # bt[p, c, s, j] = b bit for row m = c*MCW + j, k = s*128 + p
    bt = nc.dram_tensor("bt", [128, MC, KS, MCW], mybir.dt.float8e4, kind="ExternalInput")
    g = nc.dram_tensor("g", [128, KS, N_CODE], mybir.dt.float8e4, kind="ExternalInput")
    c = nc.dram_tensor("c", [M, N_CODE], mybir.dt.uint8, kind="ExternalOutput")

    dr = mybir.MatmulPerfMode.DoubleRow
    NH = N_CODE // 2

    with tile.TileContext(nc) as tc:
        with (
            tc.tile_pool(name="persist", bufs=1) as persist,
            tc.tile_pool(name="psum", bufs=4, space="PSUM") as psum_pool,
            tc.tile_pool(name="mids", bufs=8) as mids,
            tc.tile_pool(name="c8s", bufs=8) as c8s,
        ):
            g_tiles = [
                persist.tile([128, 2, N_CODE], mybir.dt.float8e4, name=f"gt{kp}", tag=f"g{kp}")
                for kp in range(KP)
            ]
            b_groups = [
                persist.tile([128, BG, KS, MCW], mybir.dt.float8e4, name=f"bg{i}", tag=f"bg{i}")
                for i in range(NBG)
            ]

            def gq(kp, q, eng):
                # one 512-col quarter of one kp pair of G (128 KiB)
                eng.dma_start(
                    out=g_tiles[kp][:, :, q * 512 : (q + 1) * 512],
                    in_=g[:, 2 * kp : 2 * kp + 2, q * 512 : (q + 1) * 512],
                )

            def gh1(kp, eng):
                # n-half 1 of one kp pair (256 KiB), needed only in phase 1
                eng.dma_start(
                    out=g_tiles[kp][:, :, NH:],
                    in_=g[:, 2 * kp : 2 * kp + 2, NH:],
                )

            def bc(ch, eng):
                # one 256-row b chunk (256 KiB) feeding m-tiles 2ch, 2ch+1
                eng.dma_start(
                    out=b_groups[ch // BG][:, ch % BG : ch % BG + 1],
                    in_=bt[:, ch : ch + 1],
                )

            # --- input pushes, consumption-ordered. Critical path for the
            # first PSUM bank: b chunk 0 (split so kp0/kp1 land first) on
            # scalar, G q0 quarters striped kp-wise across sync/gpsimd.
            nc.scalar.dma_start(
                out=b_groups[0][:, 0:1, 0:4], in_=bt[:, 0:1, 0:4]
            )
            gq(0, 0, nc.sync)
            gq(1, 0, nc.gpsimd)
            nc.scalar.dma_start(
                out=b_groups[0][:, 0:1, 4:8], in_=bt[:, 0:1, 4:8]
            )
            gq(2, 0, nc.sync)
            gq(3, 0, nc.gpsimd)
            bc(1, nc.scalar)
            gq(0, 1, nc.sync)
            gq(1, 1, nc.gpsimd)
            gq(2, 1, nc.sync)
            gq(3, 1, nc.gpsimd)
            for ch in (3, 5, 7, 9):
                bc(ch, nc.sync)
            for ch in (2, 4, 6, 8, 10):
                bc(ch, nc.gpsimd)
            gh1(0, nc.sync)
            gh1(2, nc.sync)
            gh1(1, nc.gpsimd)
            gh1(3, nc.gpsimd)
            for ch in (11, 13, 15):
                bc(ch, nc.sync)
            for ch in (12, 14):
                bc(ch, nc.gpsimd)

            # --- PE warmups: full-width 512-col matmuls on a zeroed dummy
            # tile into a dead PSUM bank. A tiny tile memsets first so the
            # earliest warmups start ~6.8us; the 512-col ones drive the
            # DVFS ramp while the first input DMAs fly.
            zw0 = persist.tile([128, 2, 128], mybir.dt.float8e4, name="zw0")
            zw = persist.tile([128, 2, 512], mybir.dt.float8e4, name="zwarm")
            nc.vector.memset(zw0, 0)
            nc.vector.memset(zw, 0)
            ps_warm = psum_pool.tile([128, NH], mybir.dt.float32, name="ps")

            def warm(cols=512):
                src = zw0 if cols <= 128 else zw
                nc.tensor.matmul(
                    ps_warm[:, 0:cols],
                    src[:, :, 0:128],
                    src[:, :, 0:cols],
                    start=True,
                    stop=True,
                    perf_mode=dr,
                )

            for _ in range(4):
                warm(64)
            for _ in range(4):
                warm(512)

            # output viewed per m-tile: m = mt*128 + p
            c_view = c.rearrange("(mt p) n -> mt p n", p=128)

            out_eng = [nc.gpsimd, nc.sync, nc.scalar]

            def bsta(mt, kp):
                mc, j = mt // 2, mt % 2
                return b_groups[mc // BG][
                    :, mc % BG, 2 * kp : 2 * kp + 2, j * 128 : (j + 1) * 128
                ]

            def extract(mid, c8, ps, s0, s1):
                nc.scalar.activation(
                    mid[:, s0:s1], ps, mybir.ActivationFunctionType.Copy
                )
                nc.vector.tensor_scalar(
                    out=mid[:, s0:s1], in0=mid[:, s0:s1], scalar1=1,
                    scalar2=None, op0=mybir.AluOpType.bitwise_and,
                )
                nc.vector.tensor_scalar(
                    out=c8[:, s0:s1], in0=mid[:, s0:s1], scalar1=0,
                    scalar2=None, op0=mybir.AluOpType.bypass,
                )

            for ph in range(2):
                n0 = ph * NH
                for mt in range(MT):
                    head_mode = ph == 0 and mt < 2
                    quarter_mode = ph == 1 and mt >= MT - 2
                    if head_mode:
                        # per-quarter PSUM banks, kp-ordered to match DMA
                        # arrival; warmup fillers bridge the data seams
                        mid = mids.tile([128, NH], mybir.dt.uint16)
                        c8 = c8s.tile([128, NH], mybir.dt.uint8)
                        for q in range(2):
                            psq = psum_pool.tile([128, 512], mybir.dt.float32, name="ps")
                            for kp in range(KP):
                                nc.tensor.matmul(
                                    psq,
                                    bsta(mt, kp),
                                    g_tiles[kp][:, :, q * 512 : (q + 1) * 512],
                                    start=(kp == 0),
                                    stop=(kp == KP - 1),
                                    perf_mode=dr,
                                )
                                if mt == 0 and q == 0 and kp == 1:
                                    warm(512)  # fill the kp1->kp2 data seam
                            extract(mid, c8, psq, q * 512, (q + 1) * 512)
                        if mt == 0:
                            warm(512)  # fill the q0->q1 data seam
                        nc.gpsimd.dma_start(out=c_view[mt][:, 0:NH], in_=c8)
                    elif not quarter_mode:
                        ps = psum_pool.tile([128, NH], mybir.dt.float32, name="ps")
                        for kp in range(KP):
                            for nt in range(2):
                                nc.tensor.matmul(
                                    ps[:, nt * 512 : (nt + 1) * 512],
                                    bsta(mt, kp),
                                    g_tiles[kp][:, :, n0 + nt * 512 : n0 + (nt + 1) * 512],
                                    start=(kp == 0),
                                    stop=(kp == KP - 1),
                                    perf_mode=dr,
                                )
                        mid = mids.tile([128, NH], mybir.dt.uint16)
                        c8 = c8s.tile([128, NH], mybir.dt.uint8)
                        extract(mid, c8, ps, 0, NH)
                        out_eng[(ph * MT + mt) % 3].dma_start(
                            out=c_view[mt][:, n0 : n0 + NH], in_=c8
                        )
                    else:
                        # final two half-tiles: per-bank PSUM quarters so the
                        # tail is one 512-col extract chain + one 64 KiB DMA
                        # on queues that have gone idle
                        mid = mids.tile([128, NH], mybir.dt.uint16)
                        c8 = c8s.tile([128, NH], mybir.dt.uint8)
                        qrings = {(MT - 2, 0): nc.gpsimd, (MT - 2, 1): nc.sync,
                                  (MT - 1, 0): nc.sync, (MT - 1, 1): nc.scalar}
                        for nt in range(2):
                            psq = psum_pool.tile([128, 512], mybir.dt.float32, name="ps")
                            for kp in range(KP):
                                nc.tensor.matmul(
                                    psq,
                                    bsta(mt, kp),
                                    g_tiles[kp][:, :, n0 + nt * 512 : n0 + (nt + 1) * 512],
                                    start=(kp == 0),
                                    stop=(kp == KP - 1),
                                    perf_mode=dr,
                                )
                            s0, s1 = nt * 512, (nt + 1) * 512
                            extract(mid, c8, psq, s0, s1)
                            qrings[(mt, nt)].dma_start(
                                out=c_view[mt][:, n0 + s0 : n0 + s1],
                                in_=c8[:, s0:s1],
                            )

    nc.finalize()
    return nc


def _get_nc():
    global _NC_CACHE
    if _NC_CACHE is None:
        _NC_CACHE = _build_bass()
    return _NC_CACHE


def _pack_inputs(b, G):
    b8 = np.asarray(b).astype(np.uint8)
    G8 = np.asarray(G).astype(np.uint8)
    # g[p, s, n], k = s*128 + p
    g_f8 = G8.reshape(KS, 128, N_CODE).transpose(1, 0, 2).astype(F8, order="C")
    bts = []
    for core in range(NCORES):
        sh = b8[core * M : (core + 1) * M]  # [M, K]
        # bt[p, c, s, j]: m = c*MCW + j, k = s*128 + p
        btc = sh.reshape(MC, MCW, KS, 128).transpose(3, 0, 2, 1)
        bts.append(btc.astype(F8, order="C"))
    return bts, g_f8


def kernel(b, G, trace=False, **run_kwargs):
    from concourse.bass_utils import run_bass_kernel_spmd

    nc = _get_nc()
    bts, g_f8 = _pack_inputs(b, G)
    in_maps = [{"bt": bts[i], "g": g_f8} for i in range(NCORES)]
    res = run_bass_kernel_spmd(
        nc, in_maps, core_ids=list(range(NCORES)), trace=trace, **run_kwargs
    )
    out = np.concatenate([res.results[i]["c"] for i in range(NCORES)], axis=0)
    out = out.astype(np.int32)
    if trace:
        kernel.last_results = res
    return out


kernel.last_results = None
